# revision 15
# baseline (speedup 1.0000x reference)
"""Trainium2 Bass kernel for nn_Memory_72756745994889 (scatter_memory).

Data-parallel over batch B=8 across 8 NeuronCores (batch b -> core b).
Per core: L2-normalize query over channels, score matmul (fp32),
softmax-over-memory (unshifted exp, safe since |score| <= ~25), top-2 via
DVE max8, read/concat matmul (f32r), update matmul (f32r) using the
identity score_query/max_sq = exp(score)/max_n exp(score) (softmax
denominator cancels), pos-row gather via indirect DMA, d_neg via the
scalar identity ||q-k||^2 = ||q||^2 + ||k||^2 - 2 q.k.  The sequential
memory-normalize loop runs on-device after an AllGather of query_update,
row-sharded 64 rows/core.
"""
import os
import sys
import numpy as np

for _p in ("/root/.axon_site", "/root/.axon_site/_ro/trn_rl_repo",
           "/root/.axon_site/_ro/pypackages", "/opt/trn_rl_repo", "/opt/pypackages"):
    if os.path.isdir(_p) and _p not in sys.path:
        sys.path.append(_p)

import concourse.bass as bass
import concourse.bacc as bacc_mod
import concourse.mybir as mybir
import concourse.tile as tile
from concourse.bass import ts
from concourse.bass_utils import run_bass_kernel_spmd
from concourse.masks import make_identity

P = 128
D = 512          # channels (= C = memory dim)
M = 512          # memory slots
N = 4096         # H*W
NCHUNK = 512     # n processed per chunk
NCH = N // NCHUNK            # 8 chunks
NSUB = NCHUNK // P           # 4 subtiles of 128 n per chunk
DC = D // P                  # 4 d-subtiles
MC = M // P                  # 4 m-subtiles
NCORES = 8
SHARD = M // NCORES          # 64 memory rows per core for the scan
F32 = mybir.dt.float32
F32R = mybir.dt.float32r
EPS_T = 1e-6                 # triplet eps
EPS_N = 1e-12                # l2norm eps

_NC_CACHE = {}


def _install_ntff_hook():
    """Provide antenv.axon_hooks if the image lacks it (profiling only)."""
    import types
    import antenv
    try:
        from antenv.axon_hooks import get_axon_ntff_profile_hook  # noqa
        return
    except ImportError:
        pass
    from trn_agent_boot.trn_boot import _ntff_profile_via_ctypes
    hook = _ntff_profile_via_ctypes("/opt/axon/libaxon_pjrt.so")
    mod = types.ModuleType("antenv.axon_hooks")
    mod._hook = hook
    mod.get_axon_ntff_profile_hook = lambda: mod._hook
    mod.set_axon_ntff_profile_hook = lambda h: setattr(mod, "_hook", h)
    sys.modules["antenv.axon_hooks"] = mod
    antenv.axon_hooks = mod


def _build_nc():
    STAGE = int(os.environ.get("KERNEL_STAGE", "9"))
    nc = bacc_mod.Bacc()
    q_d = nc.declare_dram_parameter("q", [D, N], F32, isOutput=False)
    keys_d = nc.declare_dram_parameter("keys", [M, D], F32, isOutput=False)
    kshard_d = nc.declare_dram_parameter("kshard", [SHARD, D], F32, isOutput=False)
    rowidx_d = nc.declare_dram_parameter("rowidx", [SHARD, NCORES], mybir.dt.int32,
                                         isOutput=False)
    uq_d = nc.declare_dram_parameter("uq", [2 * D, N], F32, isOutput=True)
    lc_d = nc.declare_dram_parameter("lc", [N, D], F32, isOutput=True)
    ls_d = nc.declare_dram_parameter("ls", [N], F32, isOutput=True)
    ci_d = nc.declare_dram_parameter("ci", [N], mybir.dt.int32, isOutput=True)
    mem_d = nc.declare_dram_parameter("mem_shard", [SHARD, D], F32, isOutput=True)

    qu_dram = nc.dram_tensor("qu_dram", [M, D], F32)
    qu_all = nc.dram_tensor("qu_all", [NCORES * M, D], F32, addr_space="Shared")

    AF = mybir.ActivationFunctionType
    ALU = mybir.AluOpType

    with tile.TileContext(nc) as tc:
        with (
            tc.tile_pool(name="const", bufs=1) as cpool,
            tc.tile_pool(name="persist", bufs=1) as ppool,
            tc.tile_pool(name="stream", bufs=2) as spool,
            tc.tile_pool(name="work", bufs=1) as wpool,
            tc.tile_pool(name="ps_sc", bufs=2, space="PSUM") as ps_sc,
            tc.tile_pool(name="ps_tp", bufs=2, space="PSUM") as ps_tp,
            tc.tile_pool(name="ps_misc", bufs=1, space="PSUM") as ps_misc,
            tc.tile_pool(name="ps_cm", bufs=1, space="PSUM") as ps_cm,
            tc.tile_pool(name="ps_qu", bufs=2, space="PSUM") as ps_qu,
        ):
            # ---------------- setup ----------------
            ident = cpool.tile([P, P], F32, tag="ident")
            make_identity(nc, ident[:])
            ones_f = cpool.tile([P, P], F32, tag="ones_f")
            nc.vector.memset(ones_f[:], 1.0)
            ones_r = cpool.tile([P, P], F32R, tag="ones")
            nc.vector.tensor_copy(ones_r[:], ones_f[:])
            iota_rep = cpool.tile([P, M], F32, tag="iota_rep")
            nc.gpsimd.iota(iota_rep[:], pattern=[[1, M]], base=0,
                           channel_multiplier=0,
                           allow_small_or_imprecise_dtypes=True)

            # keys native [m, d] (fp32 + f32r copies)
            k_sb = cpool.tile([P, MC, D], F32, tag="k_sb")
            nc.sync.dma_start(out=k_sb[:], in_=keys_d[:].rearrange(
                "(c p) d -> p c d", p=P))
            k_r = cpool.tile([P, MC, D], F32R, tag="k_r")
            nc.vector.tensor_copy(k_r[:], k_sb[:])

            # keysT [d, m] via PE transpose
            kT = cpool.tile([P, DC, M], F32, tag="kT")
            for mi in range(MC):
                for dc in range(DC):
                    tp = ps_tp.tile([P, P], F32, tag="tp")
                    nc.tensor.transpose(out=tp[:], in_=k_sb[:, mi, ts(dc, P)],
                                        identity=ident[:])
                    nc.scalar.copy(kT[:, dc, ts(mi, P)], tp[:])

            # kn2_rep[:, m] = sum_d keys[m,d]^2 replicated on all partitions
            kt2 = cpool.tile([P, DC, M], F32R, tag="kt2")
            nc.vector.tensor_tensor(out=kt2[:], in0=kT[:], in1=kT[:], op=ALU.mult)
            kn2_ps = ps_misc.tile([P, M], F32, tag="misc")
            for dc in range(DC):
                nc.tensor.matmul(kn2_ps[:], ones_r[:], kt2[:, dc, :],
                                 start=(dc == 0), stop=(dc == DC - 1))
            kn2_rep = cpool.tile([P, M], F32, tag="kn2")
            nc.scalar.copy(kn2_rep[:], kn2_ps[:])

            # persistent accumulators
            qu_sb = ppool.tile([P, MC, D], F32, tag="qu_sb")
            nc.vector.memset(qu_sb[:], 0.0)
            eq_run = ppool.tile([P, MC], F32, tag="eq_run")
            nc.vector.memset(eq_run[:], 0.0)
            ls_acc = ppool.tile([P, NCH * NSUB], F32, tag="ls_acc")
            ci_acc = ppool.tile([P, NCH * NSUB], mybir.dt.int32, tag="ci_acc")

            # ---------------- main loop over n-chunks ----------------
            for ch in range(NCH):
                nsl = slice(ch * NCHUNK, (ch + 1) * NCHUNK)
                # load q chunk [d(4x128), 512]
                qc = spool.tile([P, DC, NCHUNK], F32, tag="qc")
                nc.sync.dma_start(out=qc[:], in_=q_d[:, nsl].rearrange(
                    "(c p) n -> p c n", p=P))

                sqscr = wpool.tile([P, D], F32, tag="sqscr")
                # rinv replicated across partitions for layout A:
                # transpose rinv cols -> row [1, 512], then ones-matmul replicate
                # colsum-of-squares via fp32 ones-matmul -> replicated rows
                sq = wpool.tile([P, DC, NCHUNK], F32, tag="sq")
                nc.vector.tensor_tensor(out=sq[:], in0=qc[:], in1=qc[:],
                                        op=ALU.mult)
                ns_ps = ps_misc.tile([P, NCHUNK], F32, tag="misc")
                for dc in range(DC):
                    nc.tensor.matmul(ns_ps[:], ones_f[:], sq[:, dc, :],
                                     start=(dc == 0), stop=(dc == DC - 1))
                nrm_rep = wpool.tile([P, NCHUNK], F32, tag="nrm_rep")
                nc.scalar.sqrt(nrm_rep[:], ns_ps[:])
                nc.vector.tensor_scalar_max(nrm_rep[:], nrm_rep[:], EPS_N)
                rinv_rep = wpool.tile([P, NCHUNK], F32, tag="rinv_rep")
                nc.vector.reciprocal(rinv_rep[:], nrm_rep[:])

                # qn (layout A, fp32) = qc * rinv_rep ; -> updated_query rows 0:D
                qn = spool.tile([P, DC, NCHUNK], F32, tag="qn")
                for dc in range(DC):
                    nc.vector.tensor_tensor(out=qn[:, dc, :], in0=qc[:, dc, :],
                                            in1=rinv_rep[:], op=ALU.mult)
                nc.sync.dma_start(
                    out=uq_d[0:D, nsl].rearrange("(c p) n -> p c n", p=P),
                    in_=qn[:])

                # qf_B (f32r) via PE transpose of normalized qn
                qfB = [wpool.tile([P, D], F32R, tag=f"qfB{nn}", name=f"qfB{nn}")
                       for nn in range(NSUB)]
                for nn in range(NSUB):
                    for dc in range(DC):
                        tp = ps_tp.tile([P, P], F32, tag="tp")
                        nc.tensor.transpose(out=tp[:], in_=qn[:, dc, ts(nn, P)],
                                            identity=ident[:])
                        nc.scalar.copy(qfB[nn][:, ts(dc, P)], tp[:])

                if STAGE < 2:
                    continue
                # score (layout B) fp32: sB[n, m] ; per n-subtile
                sB = [wpool.tile([P, M], F32, tag=f"sB{nn}", name=f"sB{nn}") for nn in range(NSUB)]
                for nn in range(NSUB):
                    sc_ps = ps_sc.tile([P, M], F32, tag="sc")
                    for dc in range(DC):
                        nc.tensor.matmul(sc_ps[:], qn[:, dc, ts(nn, P)],
                                         kT[:, dc, :], start=(dc == 0),
                                         stop=(dc == DC - 1))
                    nc.vector.tensor_copy(sB[nn][:], sc_ps[:])

                if STAGE < 3:
                    continue
                # top-2 + masks + A' ; losses
                lsq = wpool.tile([P, NSUB], F32, tag="lsq")  # scratch d_pos parts
                apw = [wpool.tile([P, M], F32R, tag=f"apw{nn}", name=f"apw{nn}") for nn in range(NSUB)]
                scr = wpool.tile([P, M], F32, tag="scr")
                for nn in range(NSUB):
                    j = ch * NSUB + nn
                    mx = wpool.tile([P, 8], F32, tag=f"mx{nn}")
                    ix = wpool.tile([P, 8], mybir.dt.uint32, tag=f"ix{nn}")
                    nc.vector.max(out=mx[:], in_=sB[nn][:])
                    nc.vector.max_index(out=ix[:], in_max=mx[:], in_values=sB[nn][:])
                    # closest index output (uint32 -> int32 view)
                    nc.vector.tensor_copy(ci_acc[:, j:j + 1],
                                          ix[:, 0:1].bitcast(mybir.dt.int32))
                    # A' = (sB == v1) * exp(v1)
                    ev1 = wpool.tile([P, 1], F32, tag=f"ev1{nn}")
                    nc.scalar.activation(ev1[:], mx[:, 0:1], AF.Exp)
                    idxf = wpool.tile([P, 2], F32, tag=f"idxf{nn}")
                    nc.vector.tensor_copy(idxf[:], ix[:, 0:2].bitcast(
                        mybir.dt.int32))
                    nc.vector.tensor_scalar(out=apw[nn][:], in0=iota_rep[:],
                                            scalar1=idxf[:, 0:1], scalar2=ev1[:],
                                            op0=ALU.is_equal, op1=ALU.mult)
                    # kn2 at idx2: mask2 = (iota == idx2); kg2 = sum(mask2 * kn2)
                    mask2 = wpool.tile([P, M], F32, tag="mask2")
                    nc.vector.tensor_scalar(out=mask2[:], in0=iota_rep[:],
                                            scalar1=idxf[:, 1:2], scalar2=None,
                                            op0=ALU.is_equal)
                    kg2 = wpool.tile([P, 1], F32, tag=f"kg2{nn}")
                    nc.vector.tensor_tensor(out=scr[:], in0=mask2[:],
                                            in1=kn2_rep[:], op=ALU.mult)
                    nc.vector.reduce_sum(kg2[:], scr[:],
                                         axis=mybir.AxisListType.X)
                    # d_neg = sqrt(1 + kg2 - 2 v2)
                    dn2 = wpool.tile([P, 1], F32, tag=f"dn2{nn}")
                    nc.vector.tensor_scalar(out=dn2[:], in0=mx[:, 1:2],
                                            scalar1=-2.0, scalar2=1.0,
                                            op0=ALU.mult, op1=ALU.add)
                    nc.vector.tensor_add(dn2[:], dn2[:], kg2[:])
                    dn = wpool.tile([P, 1], F32, tag=f"dn{nn}")
                    nc.scalar.sqrt(dn[:], dn2[:])

                    if STAGE < 4:
                        continue
                    # pos gather + e, e^2, d_pos
                    pos_t = spool.tile([P, D], F32, tag="pos", name=f"pos{nn}", bufs=4)
                    if os.environ.get("KERNEL_GATHER", "1") == "1":
                        nc.gpsimd.indirect_dma_start(
                            out=pos_t[:], out_offset=None, in_=keys_d[:],
                            in_offset=bass.IndirectOffsetOnAxis(
                                ap=ix[:, 0:1].bitcast(mybir.dt.int32), axis=0))
                    else:
                        nc.sync.dma_start(out=pos_t[:], in_=keys_d[0:P, :])
                    e_t = wpool.tile([P, D], F32, tag="e_t")
                    sum_e = wpool.tile([P, 1], F32, tag=f"sume{nn}")
                    nc.vector.tensor_tensor(out=e_t[:],
                                            in0=qfB[nn][:].bitcast(F32),
                                            in1=pos_t[:], op=ALU.subtract)
                    nc.scalar.activation(sqscr[:], e_t[:], AF.Identity,
                                         accum_out=sum_e[:])
                    lc_t = spool.tile([P, D], F32, tag="lct", name=f"lct{nn}", bufs=4)
                    sum_e2 = wpool.tile([P, 1], F32, tag=f"sume2{nn}")
                    nc.scalar.activation(lc_t[:], e_t[:], AF.Square,
                                         accum_out=sum_e2[:])
                    nc.sync.dma_start(out=lc_d[ch * NCHUNK + nn * P:
                                               ch * NCHUNK + (nn + 1) * P, :],
                                      in_=lc_t[:])
                    dp2 = wpool.tile([P, 1], F32, tag=f"dp2{nn}")
                    nc.vector.tensor_scalar(out=dp2[:], in0=sum_e[:],
                                            scalar1=2.0 * EPS_T,
                                            scalar2=D * EPS_T * EPS_T,
                                            op0=ALU.mult, op1=ALU.add)
                    nc.vector.tensor_add(dp2[:], dp2[:], sum_e2[:])
                    dp = wpool.tile([P, 1], F32, tag=f"dp{nn}")
                    nc.scalar.sqrt(dp[:], dp2[:])
                    # loss_separate = relu(dp - dn + 1)
                    nc.vector.tensor_sub(lsq[:, nn:nn + 1], dp[:], dn[:])
                    nc.scalar.activation(ls_acc[:, j:j + 1], lsq[:, nn:nn + 1],
                                         AF.Relu, bias=1.0)

                if STAGE < 5:
                    continue
                # exp in layout A via transpose of sB; S_m colsum; smem; concat
                ea = [wpool.tile([P, NCHUNK], F32R, tag=f"ea{mi}", name=f"ea{mi}") for mi in range(MC)]
                for nn in range(NSUB):
                    for mi in range(MC):
                        tp = ps_tp.tile([P, P], F32, tag="tp")
                        nc.tensor.transpose(out=tp[:], in_=sB[nn][:, ts(mi, P)],
                                            identity=ident[:])
                        nc.scalar.activation(ea[mi][:, ts(nn, P)], tp[:], AF.Exp)
                # running E_q[m] = max_n exp
                for mi in range(MC):
                    eqc = wpool.tile([P, 1], F32, tag=f"eqc{mi}")
                    nc.vector.reduce_max(eqc[:], ea[mi][:].bitcast(F32),
                                         axis=mybir.AxisListType.X)
                    nc.vector.tensor_tensor(out=eq_run[:, mi:mi + 1],
                                            in0=eq_run[:, mi:mi + 1], in1=eqc[:],
                                            op=ALU.max)
                # S_m[n] colsum (replicated) and smem = ea / S_m
                sm_ps = ps_misc.tile([P, NCHUNK], F32, tag="misc")
                for mi in range(MC):
                    nc.tensor.matmul(sm_ps[:], ones_r[:], ea[mi][:],
                                     start=(mi == 0), stop=(mi == MC - 1))
                r_rep = wpool.tile([P, NCHUNK], F32, tag="r_rep")
                nc.vector.reciprocal(r_rep[:], sm_ps[:])
                smem = [wpool.tile([P, NCHUNK], F32R, tag=f"sB{mi}", name=f"smem{mi}")
                        for mi in range(MC)]
                for mi in range(MC):
                    nc.vector.tensor_tensor(out=smem[mi][:],
                                            in0=ea[mi][:].bitcast(F32),
                                            in1=r_rep[:], op=ALU.mult)
                # concat_memory chunk [d, n] = keys^T-contract-m @ smem
                for dcol in range(DC):
                    cm_ps = ps_cm.tile([P, NCHUNK], F32, tag="cm")
                    for mi in range(MC):
                        nc.tensor.matmul(cm_ps[:], k_r[:, mi, ts(dcol, P)],
                                         smem[mi][:], start=(mi == 0),
                                         stop=(mi == MC - 1))
                    cm_sb = spool.tile([P, NCHUNK], F32, tag="cm", name=f"cm{dcol}", bufs=4)
                    nc.scalar.copy(cm_sb[:], cm_ps[:])
                    nc.sync.dma_start(out=uq_d[D + dcol * P: D + (dcol + 1) * P,
                                               nsl], in_=cm_sb[:])

                if STAGE < 6:
                    continue
                # query_update accumulation: qu[m, d] += A'^T-contract-n @ qfB
                for mcol in range(MC):
                    qu_ps = ps_qu.tile([P, D], F32, tag="qu")
                    for nn in range(NSUB):
                        nc.tensor.matmul(qu_ps[:], apw[nn][:, ts(mcol, P)],
                                         qfB[nn][:], start=(nn == 0),
                                         stop=(nn == NSUB - 1))
                    nc.vector.tensor_add(qu_sb[:, mcol, :], qu_sb[:, mcol, :],
                                         qu_ps[:])

            # ---------------- finalize query_update ----------------
            if STAGE < 6:
                nc.vector.memset(qu_sb[:, 0, 0:1], 0.0)  # keep qu_sb written
            recq = ppool.tile([P, MC], F32, tag="recq")
            nc.vector.reciprocal(recq[:], eq_run[:])
            for mi in range(MC):
                quo = spool.tile([P, D], F32, tag="quo")
                nc.vector.tensor_scalar(out=quo[:], in0=qu_sb[:, mi, :],
                                        scalar1=recq[:, mi:mi + 1], scalar2=None,
                                        op0=ALU.mult)
                nc.sync.dma_start(out=qu_dram[ts(mi, P), :], in_=quo[:])

            # outputs ls / ci
            if STAGE < 4:
                nc.vector.memset(ls_acc[:], 0.0)
            if STAGE < 3:
                nc.vector.memset(ci_acc[:], 0)
            nc.sync.dma_start(out=ls_d[:].rearrange("(j p) -> p j", p=P),
                              in_=ls_acc[:])
            nc.sync.dma_start(out=ci_d[:].rearrange("(j p) -> p j", p=P),
                              in_=ci_acc[:])

            # ---------------- all-gather + sequential scan ----------------
            _mode = os.environ.get("KERNEL_SCAN", "1")
            if _mode == "0":
                mem_cp = ppool.tile([SHARD, D], F32, tag="mem_cp")
                nc.sync.dma_start(out=mem_cp[:], in_=kshard_d[:])
                nc.sync.dma_start(out=mem_d[:], in_=mem_cp[:])
            if _mode == "1":
                nc.gpsimd.collective_compute(
                    "AllGather", ALU.bypass,
                    replica_groups=[list(range(NCORES))],
                    ins=[qu_dram[:]], outs=[qu_all[:]])

                ridx = ppool.tile([SHARD, NCORES], mybir.dt.int32, tag="ridx")
                nc.sync.dma_start(out=ridx[:], in_=rowidx_d[:])
                mem_a = ppool.tile([SHARD, D], F32, tag="mem_a")
                mem_b = ppool.tile([SHARD, D], F32, tag="mem_b")
                scr64 = ppool.tile([SHARD, D], F32, tag="scr64")
                nc.sync.dma_start(out=mem_a[:], in_=kshard_d[:])
                for b in range(NCORES):
                    qa = spool.tile([SHARD, D], F32, tag="qa")
                    nc.gpsimd.indirect_dma_start(
                        out=qa[:], out_offset=None, in_=qu_all[:],
                        in_offset=bass.IndirectOffsetOnAxis(ap=ridx[:, b:b + 1],
                                                            axis=0))
                    nc.vector.tensor_add(mem_b[:], mem_a[:], qa[:])
                    ssb = ppool.tile([SHARD, 1], F32, tag=f"ssb{b}")
                    nc.scalar.activation(scr64[:], mem_b[:], AF.Square,
                                         accum_out=ssb[:])
                    nrmb = ppool.tile([SHARD, 1], F32, tag=f"nrmb{b}")
                    nc.scalar.sqrt(nrmb[:], ssb[:])
                    nc.vector.tensor_scalar_max(nrmb[:], nrmb[:], EPS_N)
                    rb = ppool.tile([SHARD, 1], F32, tag=f"rb{b}")
                    nc.vector.reciprocal(rb[:], nrmb[:])
                    nc.vector.tensor_scalar(out=mem_a[:], in0=mem_b[:],
                                            scalar1=rb[:], scalar2=None,
                                            op0=ALU.mult)
                nc.sync.dma_start(out=mem_d[:], in_=mem_a[:])

    nc.finalize()
    return nc


def kernel(query, keys):
    B, C, H, W = query.shape
    assert (B, C, H, W) == (8, 512, 64, 64) and keys.shape == (512, 512)
    if "nc" not in _NC_CACHE:
        _NC_CACHE["nc"] = _build_nc()
    nc = _NC_CACHE["nc"]

    q2 = np.ascontiguousarray(query.reshape(B, C, H * W).astype(np.float32))
    keys = np.ascontiguousarray(keys.astype(np.float32))
    in_maps = []
    for c in range(NCORES):
        ridx = (np.arange(NCORES)[None, :] * M + c * SHARD
                + np.arange(SHARD)[:, None]).astype(np.int32)
        in_maps.append({
            "q": q2[c],
            "keys": keys,
            "kshard": keys[c * SHARD:(c + 1) * SHARD],
            "rowidx": ridx,
        })
    trace = os.environ.get("KERNEL_TRACE", "0") == "1"
    kw = {}
    if trace:
        _install_ntff_hook()
        kw = dict(trace=True,
                  trace_cores=[int(os.environ.get("KERNEL_TRACE_CORE", "0"))])
    res = run_bass_kernel_spmd(nc, in_maps, list(range(NCORES)), **kw)
    if trace and res.exec_time_ns is not None:
        print(f"HW exec time: {res.exec_time_ns} ns")
        if res.mean_exec_time_ns:
            print(f"HW exec mean: {res.mean_exec_time_ns:.0f} ns")
    r = res.results

    updated_query = np.stack([r[c]["uq"].reshape(2 * D, H, W)
                              for c in range(B)])
    loss_compact = np.stack([r[c]["lc"] for c in range(B)])
    loss_separate = np.stack([r[c]["ls"] for c in range(B)])
    closest = np.stack([r[c]["ci"] for c in range(B)]).astype(np.int32)
    updated_memory = np.concatenate([r[c]["mem_shard"] for c in range(B)],
                                    axis=0)
    return (updated_query, updated_memory, loss_separate, loss_compact,
            closest)
